# revision 16
# baseline (speedup 1.0000x reference)
"""Trainium2 Bass kernel: masked contrastive loss, SPMD over 8 NeuronCores.

Math (reference: CustomContrastiveLoss):
  q = l2norm(logits.reshape(N,D)); k = l2norm(labels.reshape(N,D))
  sim = q @ k.T / TAU;  valid = pad_mask;  pos = (ad_i == ad_j) & valid_i & valid_j
  loss = mean_{valid rows} [ lse_valid(sim_row) - lse_pos(sim_row) ]
  (has_pos == valid because the diagonal is always a positive for valid rows)

Strategy (v2):
  * Host sorts the valid samples by ad value (index manipulation only).
    Invalid rows/cols drop out; each row's positives live in one fixed
    column window thanks to a per-core column rotation. The band equality
    masks are also host-built (index data).
  * Rows are sharded across 8 cores; every core gets all valid labels in
    bf16, kc-split layout [KC, Vy, 128].
  * On-device per core: label norms via fused DVE square+row-sum
    (tensor_tensor_reduce), scalar-engine rsqrt, scale to unit norm
    (split DVE/GpSimd), then XBAR DMA transpose straight into the matmul
    layout - the PE never transposes and ScalarE never copies.
  * |sim| <= 1/TAU so exp(sim/TAU - 1/TAU) needs no per-row max:
    PE matmul (bf16) -> ScalarE fused exp+row-sum from PSUM; the row's
    positive window is reduced from the exp tile by one DVE
    tensor_tensor_reduce against the host mask.
  * Row norms of x fold into the exp scale; x is never normalized.
  * Device emits per-row masked (lse_all - lse_pos); host sums the 8
    [128, T] partials and divides by the valid count.
"""

import os
import sys

for _p in ("/opt/trn_rl_repo", "/root/.axon_site/_ro/trn_rl_repo"):
    if os.path.isdir(_p) and _p not in sys.path:
        sys.path.append(_p)

import numpy as np
import ml_dtypes

import concourse.bass as bass
import concourse.mybir as mybir
import concourse.tile as tile
from concourse.bass_utils import run_bass_kernel_spmd

TAU = 0.05
INV_TAU = 1.0 / TAU
P = 128
D = 256
KC = D // P
NCORES = 8
GR = 2048            # psum group width (4 banks; 2 groups in flight)
CHUNK = 512          # matmul moving width (1 psum bank)
YB = 8               # y tiles per pipeline batch
F32 = mybir.dt.float32
BF16 = mybir.dt.bfloat16
AF = mybir.ActivationFunctionType
OP = mybir.AluOpType
BF = ml_dtypes.bfloat16

# ---------------------------------------------------------------------------
# This walrus build rejects more than one sync-wait per instruction.  After
# Tile scheduling, hoist excess waits onto same-engine NOPs inserted right
# before the over-subscribed instruction (engine streams are sequential, so
# the waits still happen-before the instruction).
_MAXW = 1
_wsplit_n = [0]


def _split_excess_waits(nc):
    for f in nc.m.functions:
        for bb in f.blocks:
            insts = bb.instructions
            i = 0
            while i < len(insts):
                inst = insts[i]
                si = getattr(inst, "sync_info", None)
                if si is not None and si.on_wait and len(si.on_wait) > _MAXW:
                    waits = list(si.on_wait)
                    si.on_wait = waits[:_MAXW]
                    rest = waits[_MAXW:]
                    for j in range(0, len(rest), _MAXW):
                        _wsplit_n[0] += 1
                        nop = mybir.InstNoOp(
                            name=f"wsplit-{_wsplit_n[0]}", ins=[], outs=[]
                        )
                        nop.engine = inst.engine
                        nop.sync_info = mybir.SyncInfo(
                            on_wait=rest[j : j + _MAXW], on_update=[]
                        )
                        insts.insert(i, nop)
                        i += 1
                i += 1


def build_program(V, Vy, R, Wtot, win_starts):
    T = R // P
    nyt = Vy // P
    nbat = (nyt + YB - 1) // YB
    ngr = (Vy + GR - 1) // GR
    nc = bass.Bass("TRN2", target_bir_lowering=False, debug=False)

    # kc-split bf16 inputs (host-packed; kc blocks stacked on rows)
    xs = nc.dram_tensor("xs", [KC * R, P], BF16, kind="ExternalInput")
    ys = nc.dram_tensor("ys", [KC * Vy, P], BF16, kind="ExternalInput")
    bmask = nc.dram_tensor("bmask", [P, T, Wtot], BF16, kind="ExternalInput")
    rmask = nc.dram_tensor("rmask", [P, T], F32, kind="ExternalInput")
    rpad = nc.dram_tensor("rpad", [P, T], F32, kind="ExternalInput")
    outp = nc.dram_tensor("partial", [P, T], F32, kind="ExternalOutput")

    # first y batch needed before emitting group g's matmuls
    def need_bat(g):
        hi = min((g + 1) * GR, Vy)
        return ((hi + P * YB - 1) // (P * YB)) - 1

    # (t, g) pairs whose positive window overlaps group g: their est tiles
    # must survive until the deferred band reduces run
    band_pairs = []
    for t in range(T):
        w0 = win_starts[t]
        for g in range(ngr):
            c0 = g * GR
            wg = min(GR, Vy - c0)
            if max(w0, c0) < min(w0 + Wtot, c0 + wg):
                band_pairs.append((t, g))

    with tile.TileContext(nc) as tc:
        with (
            tc.tile_pool(name="singles", bufs=1) as singles,
            tc.tile_pool(name="ystage", bufs=2) as ystage_pool,
            tc.tile_pool(name="ysc", bufs=2) as ysc_pool,
            tc.tile_pool(name="sq", bufs=3) as sq_pool,
            tc.tile_pool(name="tiny", bufs=4) as tiny,
            tc.tile_pool(name="est", bufs=3) as est_pool,
            tc.tile_pool(name="estb", bufs=max(1, len(band_pairs))) as estb_pool,
            tc.tile_pool(name="band", bufs=2) as band_pool,
            tc.tile_pool(name="pmm", bufs=2, space="PSUM") as pmm,
        ):
            b_eps = singles.tile([P, 1], F32)
            nc.vector.memset(b_eps[:], 1e-24)
            b_shift = singles.tile([P, 1], F32)
            nc.vector.memset(b_shift[:], -INV_TAU)
            b_ln20 = singles.tile([P, 1], F32)
            nc.vector.memset(b_ln20[:], float(np.log(INV_TAU)))

            masks = singles.tile([P, T, Wtot], BF16)
            nc.gpsimd.dma_start(out=masks[:], in_=bmask.ap())
            rm_s = singles.tile([P, T], F32)
            nc.sync.dma_start(out=rm_s[:], in_=rmask.ap())
            rp_s = singles.tile([P, T], F32)
            nc.sync.dma_start(out=rp_s[:], in_=rpad.ap())

            ysT = [singles.tile([P, nyt, P], BF16, name=f"ysT{kc}")
                   for kc in range(KC)]
            qT = singles.tile([P, KC, T, P], BF16)
            qssq = singles.tile([P, T], F32)
            qs20 = singles.tile([P, T], F32)
            yssq = singles.tile([P, nyt], F32)
            sparts = singles.tile([P, T, ngr], F32)
            spos = singles.tile([P, T], F32)

            # ---- x: load kc-split, fused sumsq, fold norm into exp scale,
            #      XBAR transpose raw bf16 into matmul layout
            xstage = singles.tile([P, KC, T, P], BF16)
            for kc in range(KC):
                nc.gpsimd.dma_start(
                    out=xstage[:, kc, :, :],
                    in_=xs.ap()[kc * R : (kc + 1) * R, :].rearrange(
                        "(t p) d -> p t d", p=P),
                )
            xsq = singles.tile([P, KC, T, P], BF16)
            nc.vector.tensor_mul(out=xsq[:], in0=xstage[:], in1=xstage[:])
            # reduce (kc, c) per row tile: reorder dims so both are innermost
            xsq_r = bass.AP(
                tensor=xsq.tensor, offset=xsq.offset,
                ap=[xsq.ap[0], xsq.ap[2], xsq.ap[1], xsq.ap[3]],
            )
            nc.vector.tensor_reduce(out=qssq[:], in_=xsq_r,
                                    axis=mybir.AxisListType.XY, op=OP.add)
            lnq = tiny.tile([P, T], F32)
            nc.scalar.activation(out=lnq[:], in_=qssq[:], func=AF.Ln,
                                 bias=b_eps[:], scale=1.0)
            nc.scalar.activation(out=qs20[:], in_=lnq[:], func=AF.Exp,
                                 bias=b_ln20[:], scale=-0.5)
            for kc in range(KC):
                nc.sync.dma_start_transpose(
                    qT[:, kc, :, :], xstage[:, kc, :, :].opt()
                )

            # ---- main-loop emitter (emitted interleaved with y batches).
            # Band reduces are deferred to keep DVE from stalling behind
            # ScalarE mid-pipeline; est tiles for band groups live in estb.
            band_ests = {}

            def emit_group(g):
                c0 = g * GR
                wg = min(GR, Vy - c0)
                for t in range(T):
                    ps = pmm.tile([P, GR], F32)
                    for h in range(0, wg, CHUNK):
                        hw = min(CHUNK, wg - h)
                        for kc in range(KC):
                            nc.tensor.matmul(
                                ps[:, h : h + hw],
                                qT[:, kc, t, :],
                                ysT[kc][:, :, :].opt()[:, c0 + h : c0 + h + hw],
                                start=(kc == 0), stop=(kc == KC - 1),
                            )
                    if (t, g) in band_pairs:
                        est = estb_pool.tile([P, GR], BF16)
                        band_ests[(t, g)] = est
                    else:
                        est = est_pool.tile([P, GR], BF16)
                    nc.scalar.activation(
                        out=est[:, :wg], in_=ps[:, :wg], func=AF.Exp,
                        bias=b_shift[:], scale=qs20[:, t : t + 1],
                        accum_out=sparts[:, t, g : g + 1],
                    )

            def emit_bands():
                max_gi = max(sum(1 for (tt, _) in band_pairs if tt == t)
                             for t in range(T))
                sposr = tiny.tile([P, T, max_gi], F32, name="sposr")
                if max_gi > 1:
                    nc.vector.memset(sposr[:], 0.0)
                for t in range(T):
                    w0 = win_starts[t]
                    gi = 0
                    for g in range(ngr):
                        if (t, g) not in band_ests:
                            continue
                        c0 = g * GR
                        wg = min(GR, Vy - c0)
                        lo, hi = max(w0, c0), min(w0 + Wtot, c0 + wg)
                        scr = band_pool.tile([P, Wtot], BF16)
                        nc.vector.tensor_mul(
                            out=scr[:, : hi - lo],
                            in0=band_ests[(t, g)][:, lo - c0 : hi - c0],
                            in1=masks[:, t, lo - w0 : hi - w0],
                        )
                        nc.vector.tensor_reduce(
                            out=sposr[:, t, gi : gi + 1],
                            in_=scr[:, : hi - lo],
                            axis=mybir.AxisListType.X, op=OP.add)
                        gi += 1
                # spos = rpad + sum of per-group partials
                nc.vector.tensor_add(out=spos[:], in0=sposr[:, :, 0],
                                     in1=rp_s[:])
                for gi in range(1, sposr.shape[2]):
                    nc.vector.tensor_add(out=spos[:], in0=spos[:],
                                         in1=sposr[:, :, gi])

            # ---- y pipeline: batches of YB tiles, group matmuls interleaved
            gq = list(range(ngr))
            for b in range(nbat):
                j0 = b * YB
                nb = min(YB, nyt - j0)
                yst = ystage_pool.tile([P, KC, YB, P], BF16)
                for kc in range(KC):
                    r0 = kc * Vy + j0 * P
                    nc.gpsimd.dma_start(
                        out=yst[:, kc, :nb, :],
                        in_=ys.ap()[r0 : r0 + nb * P, :].rearrange(
                            "(b p) d -> p b d", p=P),
                    )
                ysq = sq_pool.tile([P, KC, YB, P], BF16)
                nc.vector.tensor_mul(out=ysq[:, :, :nb, :],
                                     in0=yst[:, :, :nb, :],
                                     in1=yst[:, :, :nb, :])
                # two-stage row-sum keeps the big pass in 2x mode
                ys1 = sq_pool.tile([P, KC, YB], BF16)
                with nc.allow_low_precision("128-elem partials; fp32 internal"):
                    nc.vector.tensor_reduce(out=ys1[:, :, :nb],
                                            in_=ysq[:, :, :nb, :],
                                            axis=mybir.AxisListType.X,
                                            op=OP.add)
                ys1_r = bass.AP(
                    tensor=ys1.tensor, offset=ys1.offset,
                    ap=[ys1.ap[0], [ys1.ap[2][0], nb], ys1.ap[1]],
                )
                nc.vector.tensor_reduce(out=yssq[:, j0 : j0 + nb], in_=ys1_r,
                                        axis=mybir.AxisListType.X, op=OP.add)
                lny = tiny.tile([P, YB], F32)
                nc.scalar.activation(out=lny[:, :nb],
                                     in_=yssq[:, j0 : j0 + nb], func=AF.Ln,
                                     bias=b_eps[:], scale=1.0)
                ynorm = tiny.tile([P, YB], F32)
                nc.scalar.activation(out=ynorm[:, :nb], in_=lny[:, :nb],
                                     func=AF.Exp, bias=0.0, scale=-0.5)
                ysc = ysc_pool.tile([P, KC, YB, P], BF16)
                for j in range(nb):
                    eng = nc.vector if j % 2 == 0 else nc.gpsimd
                    eng.tensor_scalar_mul(
                        ysc[:, :, j, :], yst[:, :, j, :],
                        ynorm[:, j : j + 1],
                    )
                for kc in range(KC):
                    nc.sync.dma_start_transpose(
                        ysT[kc][:, j0 : j0 + nb, :],
                        ysc[:, kc, :nb, :].opt(),
                    )
                while gq and b >= need_bat(gq[0]):
                    emit_group(gq.pop(0))
            while gq:
                emit_group(gq.pop(0))
            emit_bands()

            # ---- epilogue: per-row masked loss, host sums partials
            sall = tiny.tile([P, T], F32)
            if ngr == 1:
                nc.vector.tensor_copy(out=sall[:], in_=sparts[:, :, 0])
            else:
                nc.vector.tensor_add(out=sall[:], in0=sparts[:, :, 0],
                                     in1=sparts[:, :, 1])
                for g in range(2, ngr):
                    nc.vector.tensor_add(out=sall[:], in0=sall[:],
                                         in1=sparts[:, :, g])
            lall = tiny.tile([P, T], F32)
            nc.scalar.activation(out=lall[:], in_=sall[:], func=AF.Ln,
                                 bias=0.0, scale=1.0)
            lpos = tiny.tile([P, T], F32)
            nc.scalar.activation(out=lpos[:], in_=spos[:], func=AF.Ln,
                                 bias=0.0, scale=1.0)
            dls = tiny.tile([P, T], F32)
            nc.vector.tensor_sub(out=dls[:], in0=lall[:], in1=lpos[:])
            dlm = tiny.tile([P, T], F32)
            nc.vector.tensor_mul(out=dlm[:], in0=dls[:], in1=rm_s[:])
            nc.sync.dma_start(out=outp.ap(), in_=dlm[:])

    return nc


def _roundup(a, b):
    return (a + b - 1) // b * b


def plan(valid, ad):
    """Host-side sharding plan from the pad mask / ad ids (index math only)."""
    idx = np.nonzero(valid)[0]
    V = int(idx.size)
    if V == 0:
        return None
    order = idx[np.argsort(ad[idx], kind="stable")]
    ads = ad[order].astype(np.int64)
    R = _roundup(_roundup(V, NCORES) // NCORES, P)
    Vy = _roundup(V, P)
    T = R // P
    W = int(np.bincount(ads).max())
    Wtot = min(_roundup(2 * W + P, 32), V)
    rotate = (R - P + Wtot <= V) and Wtot < V
    if rotate:
        win_starts = tuple(min(t * P, V - Wtot) for t in range(T))
    else:
        Wtot = V
        win_starts = (0,) * T
    return dict(V=V, Vy=Vy, R=R, T=T, W=W, Wtot=Wtot, win_starts=win_starts,
                rotate=rotate, order=order, ads=ads)


def _kc_split(a_bf16):
    """[N, D] bf16 -> [KC*N, P] bf16 (kc-major column split)."""
    n = a_bf16.shape[0]
    return np.ascontiguousarray(
        a_bf16.reshape(n, KC, P).transpose(1, 0, 2)).reshape(KC * n, P)


def core_inputs(pl, x, y, c):
    """Build core c's input arrays from the plan (host indexing only)."""
    V, R, Vy, W, T, Wtot = (pl["V"], pl["R"], pl["Vy"], pl["W"], pl["T"],
                            pl["Wtot"])
    order, ads = pl["order"], pl["ads"]
    g0 = c * R
    take = order[g0 : g0 + R]
    xs = np.zeros((R, D), BF)
    xs[: take.size] = x[take].astype(BF)
    adr_flat = np.full(R, -1, np.int64)
    adr_flat[: take.size] = ads[g0 : g0 + take.size]
    n_valid = max(0, min(R, V - g0))
    rmask_flat = np.zeros(R, np.float32)
    rmask_flat[:n_valid] = 1.0
    # packed [P, T]: column t holds rows [t*P, (t+1)*P) of this core's shard
    adr = adr_flat.reshape(T, P).T
    rmask = np.ascontiguousarray(rmask_flat.reshape(T, P).T)
    rpad = np.ascontiguousarray(1.0 - rmask)
    if pl["rotate"]:
        colsel = (np.arange(V) + g0 - W) % V
    else:
        colsel = np.arange(V)
    cols = order[colsel]
    ys = np.zeros((Vy, D), BF)
    ys[:V] = y[cols].astype(BF)
    adc = ads[colsel]
    # band equality masks [P, T, Wtot]
    bmask = np.zeros((P, T, Wtot), BF)
    for t in range(T):
        w0 = pl["win_starts"][t]
        bmask[:, t, :] = (adc[w0 : w0 + Wtot][None, :] == adr[:, t][:, None])
    return {"xs": _kc_split(xs), "ys": _kc_split(ys), "bmask": bmask,
            "rmask": rmask, "rpad": rpad}


_prog_cache = {}


def _get_program(pl):
    key = (pl["V"], pl["Vy"], pl["R"], pl["Wtot"], pl["win_starts"])
    if key not in _prog_cache:
        nc = build_program(pl["V"], pl["Vy"], pl["R"], pl["Wtot"],
                           pl["win_starts"])
        _prog_cache[key] = nc
    return _prog_cache[key]


def kernel(logits, labels, pad_mask, ad_idxs, _want_results=False, **run_kwargs):
    x = np.ascontiguousarray(np.asarray(logits), dtype=np.float32).reshape(-1, D)
    y = np.ascontiguousarray(np.asarray(labels), dtype=np.float32).reshape(-1, D)
    valid = np.asarray(pad_mask).reshape(-1).astype(bool)
    ad = np.asarray(ad_idxs).reshape(-1).astype(np.int64)

    pl = plan(valid, ad)
    if pl is None:
        return np.float32(0.0)

    nc = _get_program(pl)
    # CoreSim chokes on the inserted NOPs, so split waits only for the HW path
    if not getattr(nc, "_waits_split", False):
        # populate .instr bytes for extended-ISA insts (tensor_tensor_reduce);
        # Bacc.compile does this but raw Bass does not ("ISA wrong length")
        mybir.codegen_inst_isa_subclasses(nc)
        _split_excess_waits(nc)
        nc._waits_split = True
    in_maps = [core_inputs(pl, x, y, c) for c in range(NCORES)]
    res = run_bass_kernel_spmd(nc, in_maps, core_ids=list(range(NCORES)),
                               **run_kwargs)
    total = sum(float(res.results[c]["partial"].astype(np.float64).sum())
                for c in range(NCORES))
    loss = np.float32(total / pl["V"])
    if _want_results:
        return loss, res
    return loss
